# revision 9
# baseline (speedup 1.0000x reference)
"""AudioAttNet Trainium2 kernel (restructured v3).

Computation (per batch element b of 65536):
  x[29, 8] -> conv1d(29->16, k=3, same) + lrelu(0.02)
           -> conv1d(16->8)  + lrelu
           -> conv1d(8->4)   + lrelu
           -> conv1d(4->128) + lrelu          = y [8, 128]   (seq-major)
  logits = y @ wl.T   (+bl; constant along softmax axis so it cancels)
  attn   = softmax(logits, axis=seq)
  out    = sum_seq(y * attn)                  = [128]

Mapping: pure data parallel over batch across 8 cores (8192 batches/core).

Design notes:
  * x is transposed + cast to fp16 on the HOST -> DRAM holds [232, 8192]
    per core; chunk loads are plain contiguous DMAs. No on-chip input
    transposes/converts.
  * Output ships as level-2 partial sums (u2/d2, 2 seq-groups each) in
    [d, b] layout; host does the final pair-add, divide and transpose.
    No on-chip output transposes, reciprocal, scale, or last tree level.
  * All biases are folded into matmuls via ones-row tricks (conv2's
    weight emits a constant-1.0 65th row; conv3's bias rides it; conv4
    strips keep spare rows made constant via conv3's bias column, and
    w4's bias rides those). conv3/conv4 evacuations are pure prelu.
  * PSUM-evacuation work is almost all on ACT (prelu/exp at ~1ns/elem);
    DVE does the fp16 SBUF tail (tensor_tensor hits 2x mode with flat
    contiguous APs) plus one conv4 slot for balance.
  * 3-stage pipeline: conv4+linear+exp(ch) [PE burst of 32 matmuls
    back-to-back to keep the PE HAM un-throttled] || tail(ch-1) [DVE]
    || conv1-3(ch+1) [PE+ACT ping-pong at iteration end].
  * PSUM: 2 rotating slots of [128, 2, 1024] fp32 (4 banks each).
"""

import os
import numpy as np
from contextlib import ExitStack

import concourse.bass as bass
from concourse import bacc
from concourse import mybir
from concourse.tile import TileContext
from concourse.bass_utils import run_bass_kernel_spmd

F16 = mybir.dt.float16
F32 = mybir.dt.float32
AF = mybir.ActivationFunctionType
ALU = mybir.AluOpType

B, C, S = 65536, 29, 8
CS = C * S                   # 232
NCORES = 8
BPC = B // NCORES            # 8192 batches per core
BC = 1024                    # batches per chunk
NCHUNK = BPC // BC
NEG = 0.02

MMN = 512                    # matmul moving-operand max N
NT = BC // MMN
# conv4 psum slots (0..3) whose evacuation runs on DVE instead of ACT
C4_DVE = set(int(t) for t in os.environ.get("CC_C4DVE", "3").split(",") if t != "")


def _build_nc():
    nc = bacc.Bacc()

    xhi_d = nc.declare_dram_parameter("xhi", [128, BPC], F16, isOutput=False)
    xlo_d = nc.declare_dram_parameter("xlo", [CS - 128, BPC], F16, isOutput=False)
    w1a_d = nc.declare_dram_parameter("w1a", [128, 128], F16, isOutput=False)
    w1b_d = nc.declare_dram_parameter("w1b", [CS - 128, 128], F16, isOutput=False)
    w2_d = nc.declare_dram_parameter("w2e", [128, 65], F16, isOutput=False)
    w3_d = nc.declare_dram_parameter("w3r", [65, 128], F16, isOutput=False)
    w4_d0 = nc.declare_dram_parameter("w4g0", [128, 128], F16, isOutput=False)
    w4_d1 = nc.declare_dram_parameter("w4g1", [128, 128], F16, isOutput=False)
    wl_d = nc.declare_dram_parameter("wlt", [128, 128], F16, isOutput=False)
    b1_d = nc.declare_dram_parameter("b1v", [128, 1], F32, isOutput=False)
    b2_d = nc.declare_dram_parameter("b2v", [65, 1], F32, isOutput=False)
    # level-2 partial sums, 2 seq-groups per batch: [128, NCHUNK * 2 * BC]
    outu_d = nc.declare_dram_parameter("outu", [128, BPC * 2], F16, isOutput=True)
    outd_d = nc.declare_dram_parameter("outd", [128, BPC * 2], F16, isOutput=True)

    with TileContext(nc) as tc, ExitStack() as ctx:
        consts = ctx.enter_context(tc.tile_pool(name="consts", bufs=1))
        w1a = consts.tile_from(w1a_d[:])
        w1b = consts.tile_from(w1b_d[:])
        w2e = consts.tile_from(w2_d[:])
        w3r = consts.tile_from(w3_d[:])
        w4g0 = consts.tile_from(w4_d0[:])
        w4g1 = consts.tile_from(w4_d1[:])
        w4g = [w4g0, w4g1]
        wlt = consts.tile_from(wl_d[:])
        b1v = consts.tile_from(b1_d[:])
        b2v = consts.tile_from(b2_d[:])
        alpha_v = consts.tile([128, 1], F32)
        nc.vector.memset(alpha_v[:], NEG)
        # touch the act table set early so ACT_TABLE_LOAD overlaps the
        # first input DMA instead of stalling the first conv
        warm = consts.tile([1, 1], F16)
        nc.scalar.activation(warm[:], alpha_v[0:1, :], AF.Exp)

        io = ctx.enter_context(tc.tile_pool(name="io", bufs=2))
        acts = ctx.enter_context(tc.tile_pool(name="acts", bufs=2))
        big = ctx.enter_context(tc.tile_pool(
            name="bigsb", bufs=int(os.environ.get("CC_BIGBUFS", "3"))))
        tree = ctx.enter_context(tc.tile_pool(
            name="tree", bufs=int(os.environ.get("CC_TREEBUFS", "1"))))
        psp = ctx.enter_context(tc.tile_pool(name="psp", bufs=2, space="PSUM"))

        def pslot(name):
            return psp.tile([128, 2, BC], F32, tag="ps", name=name)

        def load(ch):
            sl = slice(ch * BC, (ch + 1) * BC)
            xt1 = io.tile([128, BC], F16, tag="xt1", name=f"xt1_{ch}")
            xt2 = io.tile([CS - 128, BC], F16, tag="xt2", name=f"xt2_{ch}")
            nc.sync.dma_start(out=xt1[:], in_=xhi_d[:, sl])
            nc.sync.dma_start(out=xt2[:], in_=xlo_d[:, sl])
            return xt1, xt2

        def mm_sl(t):
            return slice(t * MMN, (t + 1) * MMN)

        def convs123(ch, xt):
            """stage A: conv1 -> conv2 -> conv3 (PE + ACT ping-pong)."""
            xt1, xt2 = xt
            y1 = acts.tile([128, BC], F16, tag="y1")
            p1 = pslot(f"p1_{ch}")
            for t in range(NT):
                nc.tensor.matmul(p1[:, 0, mm_sl(t)], w1a[:], xt1[:, mm_sl(t)],
                                 start=True, stop=False)
                nc.tensor.matmul(p1[:, 0, mm_sl(t)], w1b[:], xt2[:, mm_sl(t)],
                                 start=False, stop=True)
            nc.scalar.activation(y1[:], p1[:, 0, :], AF.Prelu,
                                 bias=b1v[:], alpha=alpha_v[:])
            y2 = acts.tile([65, BC], F16, tag="y2")
            p2 = pslot(f"p2_{ch}")
            for t in range(NT):
                nc.tensor.matmul(p2[0:65, 0, mm_sl(t)], w2e[:],
                                 y1[:, mm_sl(t)], start=True, stop=True)
            nc.scalar.activation(y2[:], p2[0:65, 0, :], AF.Prelu,
                                 bias=b2v[:], alpha=alpha_v[0:65, :])
            y3 = acts.tile([128, BC], F16, tag="y3")
            p3 = pslot(f"p3_{ch}")
            for t in range(NT):
                nc.tensor.matmul(p3[:, 0, mm_sl(t)], w3r[:],
                                 y2[:, mm_sl(t)], start=True, stop=True)
            nc.scalar.activation(y3[:], p3[:, 0, :], AF.Prelu,
                                 alpha=alpha_v[:])
            return y3

        def conv4lin(ch, y3):
            """stage B: conv4 + linear + exp. 32 back-to-back PE matmuls."""
            yy = big.tile([128, S, BC], F16, tag="yy")   # [d, s, b]
            slot_i = 0
            for g in range(2):
                for half in range(2):
                    p4 = pslot(f"p4_{ch}_{g}_{half}")
                    for jj in range(2):
                        j = 2 * half + jj
                        for t in range(NT):
                            nc.tensor.matmul(
                                p4[:, jj, mm_sl(t)],
                                w4g[g][32 * j:32 * (j + 1), :],
                                y3[32 * j:32 * (j + 1), mm_sl(t)],
                                start=True, stop=True,
                                tile_position=(32 * j, 0))
                    out_ap = yy[:, 4 * g + 2 * half:4 * g + 2 * half + 2, :]
                    if slot_i in C4_DVE:
                        z4 = acts.tile([128, 2, BC], F16, tag="z4",
                                       name=f"z4_{ch}_{slot_i}")
                        nc.vector.tensor_copy(
                            z4[:].rearrange("p a b -> p (a b)"),
                            p4[:].rearrange("p a b -> p (a b)"))
                        nc.vector.scalar_tensor_tensor(
                            out_ap.rearrange("p a b -> p (a b)"),
                            z4[:].rearrange("p a b -> p (a b)"), NEG,
                            z4[:].rearrange("p a b -> p (a b)"),
                            ALU.mult, ALU.max)
                    else:
                        nc.scalar.activation(out_ap, p4[:], AF.Prelu,
                                             alpha=alpha_v[:])
                    slot_i += 1
            ee = big.tile([128, S, BC], F16, tag="ee")   # [e, s, b]
            for q in range(4):
                pl = pslot(f"pl_{ch}_{q}")
                for jj in range(2):
                    s = 2 * q + jj
                    for t in range(NT):
                        nc.tensor.matmul(pl[:, jj, mm_sl(t)], wlt[:],
                                         yy[:, s, mm_sl(t)],
                                         start=True, stop=True)
                nc.scalar.activation(ee[:, 2 * q:2 * q + 2, :], pl[:], AF.Exp)
            return yy, ee

        def tail_product(ch, yy, ee):
            pp = big.tile([128, S, BC], F16, tag="pp")
            for h in range(2):
                nc.vector.tensor_mul(
                    pp[:, 4 * h:4 * h + 4, :].rearrange("p a b -> p (a b)"),
                    yy[:, 4 * h:4 * h + 4, :].rearrange("p a b -> p (a b)"),
                    ee[:, 4 * h:4 * h + 4, :].rearrange("p a b -> p (a b)"))
            return pp

        def tail_trees(ch, pp, ee):
            sl = slice(ch * 2 * BC, (ch + 1) * 2 * BC)
            flat = "p a b -> p (a b)"
            u1 = tree.tile([128, 4, BC], F16, tag="u1")
            nc.vector.tensor_add(u1[:].rearrange(flat),
                                 pp[:, 0:4, :].rearrange(flat),
                                 pp[:, 4:8, :].rearrange(flat))
            d1 = tree.tile([128, 4, BC], F16, tag="d1")
            nc.vector.tensor_add(d1[:].rearrange(flat),
                                 ee[:, 0:4, :].rearrange(flat),
                                 ee[:, 4:8, :].rearrange(flat))
            u2 = tree.tile([128, 2, BC], F16, tag="u2")
            nc.vector.tensor_add(u2[:].rearrange(flat),
                                 u1[:, 0:2, :].rearrange(flat),
                                 u1[:, 2:4, :].rearrange(flat))
            d2 = tree.tile([128, 2, BC], F16, tag="d2")
            nc.vector.tensor_add(d2[:].rearrange(flat),
                                 d1[:, 0:2, :].rearrange(flat),
                                 d1[:, 2:4, :].rearrange(flat))
            nc.sync.dma_start(out=outu_d[:, sl], in_=u2[:].rearrange(flat))
            nc.sync.dma_start(out=outd_d[:, sl], in_=d2[:].rearrange(flat))

        # ---- 3-stage software pipeline ----
        # iter ch issues: conv4lin(ch) | tail(ch-1) | convs123(ch+1).
        # PE sees the 32-matmul burst first, then the conv chain of the
        # next chunk; DVE sees the ready tail product first.
        xt = load(0)
        xt1n = load(1)
        y3_cur = convs123(0, xt)
        prev = None
        for ch in range(NCHUNK):
            if ch + 2 < NCHUNK:
                xt_next = load(ch + 2)
            else:
                xt_next = None
            if prev is not None:
                pp = tail_product(prev[0], prev[1], prev[2])
            cur = conv4lin(ch, y3_cur)
            if prev is not None:
                tail_trees(prev[0], pp, prev[2])
            if ch + 1 < NCHUNK:
                y3_cur = convs123(ch + 1, xt1n)
                xt1n = xt_next
            prev = (ch, *cur)
        pp = tail_product(prev[0], prev[1], prev[2])
        tail_trees(prev[0], pp, prev[2])

    nc.compile()
    return nc


def _win(s):
    return {s3 for s3 in (s - 1, s, s + 1) if 0 <= s3 < S}


def _host_weights(w1, b1, w2, b2, w3, b3, w4, b4, wl):
    def eff(wc, cin, cout):
        m = np.zeros((cin * S, cout * S), np.float32)
        for co in range(cout):
            for ci in range(cin):
                for k in range(3):
                    for so in range(S):
                        si = so + k - 1
                        if 0 <= si < S:
                            m[ci * S + si, co * S + so] = wc[co, ci, k]
        return m

    w1e = eff(w1, 29, 16)                        # [232, 128]
    w2e = np.zeros((128, 65), np.float32)
    w2e[:, :64] = eff(w2, 16, 8)
    b2v = np.concatenate([np.repeat(b2, S), [1.0]]).astype(np.float32)

    eff3 = eff(w3, 8, 4)                         # [64, 32]
    w3r = np.zeros((65, 128), np.float32)
    w4g = np.zeros((2, 128, 128), np.float32)
    for j in range(4):
        s3set = sorted(_win(j) | _win(4 + j))
        rows = [(c3, s3) for s3 in s3set for c3 in range(4)]
        ones_idx = len(rows)
        for r, (c3, s3) in enumerate(rows):
            w3r[0:64, 32 * j + r] = eff3[:, c3 * S + s3]
            w3r[64, 32 * j + r] = b3[c3]
        w3r[64, 32 * j + ones_idx] = 1.0
        for g in range(2):
            s = 4 * g + j
            for r, (c3, s3) in enumerate(rows):
                k = s3 - s + 1
                if 0 <= k < 3:
                    w4g[g, 32 * j + r, :] = w4[:, c3, k]
            w4g[g, 32 * j + ones_idx, :] = b4

    return dict(
        w1a=w1e[:128].astype(np.float16),
        w1b=w1e[128:].astype(np.float16),
        w2e=w2e.astype(np.float16),
        w3r=w3r.astype(np.float16),
        w4g0=np.ascontiguousarray(w4g[0]).astype(np.float16),
        w4g1=np.ascontiguousarray(w4g[1]).astype(np.float16),
        wlt=np.ascontiguousarray(wl.T).astype(np.float16),
        b1v=np.repeat(b1, S).reshape(128, 1).astype(np.float32),
        b2v=b2v.reshape(65, 1).astype(np.float32),
    )


def make_in_maps(inputs):
    """Full-input dict -> per-core in_maps (host-side transpose + fp16)."""
    x = np.asarray(inputs["x"], np.float32).reshape(B, CS)
    xt = np.ascontiguousarray(x.astype(np.float16).T)       # [232, B]
    wmap = _host_weights(
        *[np.asarray(inputs[k], np.float32) for k in
          ("w1", "b1", "w2", "b2", "w3", "b3", "w4", "b4", "wl")])
    in_maps = []
    for i in range(NCORES):
        sl = slice(i * BPC, (i + 1) * BPC)
        m = {"xhi": np.ascontiguousarray(xt[:128, sl]),
             "xlo": np.ascontiguousarray(xt[128:, sl])}
        m.update(wmap)
        in_maps.append(m)
    return in_maps


_NC_CACHE = None


def kernel(x, w1, b1, w2, b2, w3, b3, w4, b4, wl, bl):
    global _NC_CACHE
    # bl is constant along the softmax axis -> cancels; intentionally unused.
    in_maps = make_in_maps(dict(x=x, w1=w1, b1=b1, w2=w2, b2=b2, w3=w3,
                                b3=b3, w4=w4, b4=b4, wl=wl))
    if _NC_CACHE is None:
        _NC_CACHE = _build_nc()
    nc = _NC_CACHE

    core_ids = list(range(NCORES))
    res = run_bass_kernel_spmd(nc, in_maps, core_ids)
    outs = []
    for i in range(NCORES):
        # [128, NCHUNK, 2, BC] level-2 partials -> sum pairs, divide, transpose
        u2 = res.results[i]["outu"].astype(np.float32).reshape(128, NCHUNK, 2, BC)
        d2 = res.results[i]["outd"].astype(np.float32).reshape(128, NCHUNK, 2, BC)
        uu = u2.sum(axis=2).reshape(128, BPC)
        dd = d2.sum(axis=2).reshape(128, BPC)
        outs.append((uu / dd).T)
    return np.ascontiguousarray(np.concatenate(outs, axis=0))


# revision 11
# speedup vs baseline: 1.0002x; 1.0002x over previous
"""AudioAttNet Trainium2 kernel (restructured v3).

Computation (per batch element b of 65536):
  x[29, 8] -> conv1d(29->16, k=3, same) + lrelu(0.02)
           -> conv1d(16->8)  + lrelu
           -> conv1d(8->4)   + lrelu
           -> conv1d(4->128) + lrelu          = y [8, 128]   (seq-major)
  logits = y @ wl.T   (+bl; constant along softmax axis so it cancels)
  attn   = softmax(logits, axis=seq)
  out    = sum_seq(y * attn)                  = [128]

Mapping: pure data parallel over batch across 8 cores (8192 batches/core).

Design notes:
  * x is transposed + cast to fp16 on the HOST -> DRAM holds [232, 8192]
    per core; chunk loads are plain contiguous DMAs. No on-chip input
    transposes/converts.
  * Output ships as level-2 partial sums (u2/d2, 2 seq-groups each) in
    [d, b] layout; host does the final pair-add, divide and transpose.
    No on-chip output transposes, reciprocal, scale, or last tree level.
  * All biases are folded into matmuls via ones-row tricks (conv2's
    weight emits a constant-1.0 65th row; conv3's bias rides it; conv4
    strips keep spare rows made constant via conv3's bias column, and
    w4's bias rides those). conv3/conv4 evacuations are pure prelu.
  * PSUM-evacuation work is almost all on ACT (prelu/exp at ~1ns/elem);
    DVE does the fp16 SBUF tail (tensor_tensor hits 2x mode with flat
    contiguous APs) plus one conv4 slot for balance.
  * 3-stage pipeline: conv4+linear+exp(ch) [PE burst of 32 matmuls
    back-to-back to keep the PE HAM un-throttled] || tail(ch-1) [DVE]
    || conv1-3(ch+1) [PE+ACT ping-pong at iteration end].
  * PSUM: 2 rotating slots of [128, 2, 1024] fp32 (4 banks each).
"""

import os
import numpy as np
from contextlib import ExitStack

import concourse.bass as bass
from concourse import bacc
from concourse import mybir
from concourse.tile import TileContext
from concourse.bass_utils import run_bass_kernel_spmd

F16 = mybir.dt.float16
F32 = mybir.dt.float32
AF = mybir.ActivationFunctionType
ALU = mybir.AluOpType

B, C, S = 65536, 29, 8
CS = C * S                   # 232
NCORES = 8
BPC = B // NCORES            # 8192 batches per core
BC = 1024                    # batches per chunk
NCHUNK = BPC // BC
NEG = 0.02

MMN = 512                    # matmul moving-operand max N
NT = BC // MMN
# conv4 psum slots (0..3) whose evacuation runs on DVE instead of ACT
C4_DVE = set(int(t) for t in os.environ.get("CC_C4DVE", "3").split(",") if t != "")


def _build_nc():
    nc = bacc.Bacc()

    xhi_d = nc.declare_dram_parameter("xhi", [128, BPC], F16, isOutput=False)
    xlo_d = nc.declare_dram_parameter("xlo", [CS - 128, BPC], F16, isOutput=False)
    w1a_d = nc.declare_dram_parameter("w1a", [128, 128], F16, isOutput=False)
    w1b_d = nc.declare_dram_parameter("w1b", [CS - 128, 128], F16, isOutput=False)
    w2_d = nc.declare_dram_parameter("w2e", [128, 65], F16, isOutput=False)
    w3_d = nc.declare_dram_parameter("w3r", [65, 128], F16, isOutput=False)
    w4_d0 = nc.declare_dram_parameter("w4g0", [128, 128], F16, isOutput=False)
    w4_d1 = nc.declare_dram_parameter("w4g1", [128, 128], F16, isOutput=False)
    wl_d = nc.declare_dram_parameter("wlt", [128, 128], F16, isOutput=False)
    b1_d = nc.declare_dram_parameter("b1v", [128, 1], F32, isOutput=False)
    b2_d = nc.declare_dram_parameter("b2v", [65, 1], F32, isOutput=False)
    # level-2 partial sums, 2 seq-groups per batch: [128, NCHUNK * 2 * BC]
    outu_d = nc.declare_dram_parameter("outu", [128, BPC * 2], F16, isOutput=True)
    outd_d = nc.declare_dram_parameter("outd", [128, BPC * 2], F16, isOutput=True)

    with TileContext(nc) as tc, ExitStack() as ctx:
        consts = ctx.enter_context(tc.tile_pool(name="consts", bufs=1))
        w1a = consts.tile_from(w1a_d[:])
        w1b = consts.tile_from(w1b_d[:])
        w2e = consts.tile_from(w2_d[:])
        w3r = consts.tile_from(w3_d[:])
        w4g0 = consts.tile_from(w4_d0[:])
        w4g1 = consts.tile_from(w4_d1[:])
        w4g = [w4g0, w4g1]
        wlt = consts.tile_from(wl_d[:])
        b1v = consts.tile_from(b1_d[:])
        b2v = consts.tile_from(b2_d[:])
        alpha_v = consts.tile([128, 1], F32)
        nc.vector.memset(alpha_v[:], NEG)
        # touch the act table set early so ACT_TABLE_LOAD overlaps the
        # first input DMA instead of stalling the first conv
        warm = consts.tile([1, 1], F16)
        nc.scalar.activation(warm[:], alpha_v[0:1, :], AF.Exp)

        io = ctx.enter_context(tc.tile_pool(name="io", bufs=2))
        acts = ctx.enter_context(tc.tile_pool(name="acts", bufs=2))
        big = ctx.enter_context(tc.tile_pool(
            name="bigsb", bufs=int(os.environ.get("CC_BIGBUFS", "3"))))
        bigp = ctx.enter_context(tc.tile_pool(
            name="bigp", bufs=int(os.environ.get("CC_PPBUFS", "2"))))
        tree = ctx.enter_context(tc.tile_pool(
            name="tree", bufs=int(os.environ.get("CC_TREEBUFS", "2"))))
        psp = ctx.enter_context(tc.tile_pool(name="psp", bufs=2, space="PSUM"))

        def pslot(name):
            return psp.tile([128, 2, BC], F32, tag="ps", name=name)

        def load(ch):
            sl = slice(ch * BC, (ch + 1) * BC)
            xt1 = io.tile([128, BC], F16, tag="xt1", name=f"xt1_{ch}")
            xt2 = io.tile([CS - 128, BC], F16, tag="xt2", name=f"xt2_{ch}")
            nc.sync.dma_start(out=xt1[:], in_=xhi_d[:, sl])
            nc.sync.dma_start(out=xt2[:], in_=xlo_d[:, sl])
            return xt1, xt2

        def mm_sl(t):
            return slice(t * MMN, (t + 1) * MMN)

        def convs123(ch, xt):
            """stage A: conv1 -> conv2 -> conv3 (PE + ACT ping-pong)."""
            xt1, xt2 = xt
            y1 = acts.tile([128, BC], F16, tag="y1")
            p1 = pslot(f"p1_{ch}")
            for t in range(NT):
                nc.tensor.matmul(p1[:, 0, mm_sl(t)], w1a[:], xt1[:, mm_sl(t)],
                                 start=True, stop=False)
                nc.tensor.matmul(p1[:, 0, mm_sl(t)], w1b[:], xt2[:, mm_sl(t)],
                                 start=False, stop=True)
            nc.scalar.activation(y1[:], p1[:, 0, :], AF.Prelu,
                                 bias=b1v[:], alpha=alpha_v[:])
            y2 = acts.tile([65, BC], F16, tag="y2")
            p2 = pslot(f"p2_{ch}")
            for t in range(NT):
                nc.tensor.matmul(p2[0:65, 0, mm_sl(t)], w2e[:],
                                 y1[:, mm_sl(t)], start=True, stop=True)
            nc.scalar.activation(y2[:], p2[0:65, 0, :], AF.Prelu,
                                 bias=b2v[:], alpha=alpha_v[0:65, :])
            y3 = acts.tile([128, BC], F16, tag="y3")
            p3 = pslot(f"p3_{ch}")
            for t in range(NT):
                nc.tensor.matmul(p3[:, 0, mm_sl(t)], w3r[:],
                                 y2[:, mm_sl(t)], start=True, stop=True)
            nc.scalar.activation(y3[:], p3[:, 0, :], AF.Prelu,
                                 alpha=alpha_v[:])
            return y3

        def conv4lin(ch, y3):
            """stage B: conv4 + linear + exp. 32 back-to-back PE matmuls."""
            yy = big.tile([128, S, BC], F16, tag="yy")   # [d, s, b]
            slot_i = 0
            for g in range(2):
                for half in range(2):
                    p4 = pslot(f"p4_{ch}_{g}_{half}")
                    for jj in range(2):
                        j = 2 * half + jj
                        for t in range(NT):
                            nc.tensor.matmul(
                                p4[:, jj, mm_sl(t)],
                                w4g[g][32 * j:32 * (j + 1), :],
                                y3[32 * j:32 * (j + 1), mm_sl(t)],
                                start=True, stop=True,
                                tile_position=(32 * j, 0))
                    out_ap = yy[:, 4 * g + 2 * half:4 * g + 2 * half + 2, :]
                    if slot_i in C4_DVE:
                        z4 = acts.tile([128, 2, BC], F16, tag="z4",
                                       name=f"z4_{ch}_{slot_i}")
                        nc.vector.tensor_copy(
                            z4[:].rearrange("p a b -> p (a b)"),
                            p4[:].rearrange("p a b -> p (a b)"))
                        nc.vector.scalar_tensor_tensor(
                            out_ap.rearrange("p a b -> p (a b)"),
                            z4[:].rearrange("p a b -> p (a b)"), NEG,
                            z4[:].rearrange("p a b -> p (a b)"),
                            ALU.mult, ALU.max)
                    else:
                        nc.scalar.activation(out_ap, p4[:], AF.Prelu,
                                             alpha=alpha_v[:])
                    slot_i += 1
            ee = big.tile([128, S, BC], F16, tag="ee")   # [e, s, b]
            for q in range(4):
                pl = pslot(f"pl_{ch}_{q}")
                for jj in range(2):
                    s = 2 * q + jj
                    for t in range(NT):
                        nc.tensor.matmul(pl[:, jj, mm_sl(t)], wlt[:],
                                         yy[:, s, mm_sl(t)],
                                         start=True, stop=True)
                nc.scalar.activation(ee[:, 2 * q:2 * q + 2, :], pl[:], AF.Exp)
            return yy, ee

        def tail_product(ch, yy, ee):
            pp = bigp.tile([128, S, BC], F16, tag="pp")
            for h in range(2):
                nc.vector.tensor_mul(
                    pp[:, 4 * h:4 * h + 4, :].rearrange("p a b -> p (a b)"),
                    yy[:, 4 * h:4 * h + 4, :].rearrange("p a b -> p (a b)"),
                    ee[:, 4 * h:4 * h + 4, :].rearrange("p a b -> p (a b)"))
            return pp

        def tail_trees(ch, pp, ee):
            sl = slice(ch * 2 * BC, (ch + 1) * 2 * BC)
            flat = "p a b -> p (a b)"
            u1 = tree.tile([128, 4, BC], F16, tag="u1")
            nc.vector.tensor_add(u1[:].rearrange(flat),
                                 pp[:, 0:4, :].rearrange(flat),
                                 pp[:, 4:8, :].rearrange(flat))
            d1 = tree.tile([128, 4, BC], F16, tag="d1")
            nc.vector.tensor_add(d1[:].rearrange(flat),
                                 ee[:, 0:4, :].rearrange(flat),
                                 ee[:, 4:8, :].rearrange(flat))
            u2 = tree.tile([128, 2, BC], F16, tag="u2")
            nc.vector.tensor_add(u2[:].rearrange(flat),
                                 u1[:, 0:2, :].rearrange(flat),
                                 u1[:, 2:4, :].rearrange(flat))
            d2 = tree.tile([128, 2, BC], F16, tag="d2")
            nc.vector.tensor_add(d2[:].rearrange(flat),
                                 d1[:, 0:2, :].rearrange(flat),
                                 d1[:, 2:4, :].rearrange(flat))
            nc.sync.dma_start(out=outu_d[:, sl], in_=u2[:].rearrange(flat))
            nc.sync.dma_start(out=outd_d[:, sl], in_=d2[:].rearrange(flat))

        # ---- 3-stage software pipeline ----
        # iter ch issues: conv4lin(ch) | tail(ch-1) | convs123(ch+1).
        # PE sees the 32-matmul burst first, then the conv chain of the
        # next chunk; DVE sees the ready tail product first.
        xt = load(0)
        xt1n = load(1)
        y3_cur = convs123(0, xt)
        prev = None
        for ch in range(NCHUNK):
            if ch + 2 < NCHUNK:
                xt_next = load(ch + 2)
            else:
                xt_next = None
            if prev is not None:
                pp = tail_product(prev[0], prev[1], prev[2])
            cur = conv4lin(ch, y3_cur)
            if prev is not None:
                tail_trees(prev[0], pp, prev[2])
            if ch + 1 < NCHUNK:
                y3_cur = convs123(ch + 1, xt1n)
                xt1n = xt_next
            prev = (ch, *cur)
        pp = tail_product(prev[0], prev[1], prev[2])
        tail_trees(prev[0], pp, prev[2])

    nc.compile()
    return nc


def _win(s):
    return {s3 for s3 in (s - 1, s, s + 1) if 0 <= s3 < S}


def _host_weights(w1, b1, w2, b2, w3, b3, w4, b4, wl):
    def eff(wc, cin, cout):
        m = np.zeros((cin * S, cout * S), np.float32)
        for co in range(cout):
            for ci in range(cin):
                for k in range(3):
                    for so in range(S):
                        si = so + k - 1
                        if 0 <= si < S:
                            m[ci * S + si, co * S + so] = wc[co, ci, k]
        return m

    w1e = eff(w1, 29, 16)                        # [232, 128]
    w2e = np.zeros((128, 65), np.float32)
    w2e[:, :64] = eff(w2, 16, 8)
    b2v = np.concatenate([np.repeat(b2, S), [1.0]]).astype(np.float32)

    eff3 = eff(w3, 8, 4)                         # [64, 32]
    w3r = np.zeros((65, 128), np.float32)
    w4g = np.zeros((2, 128, 128), np.float32)
    for j in range(4):
        s3set = sorted(_win(j) | _win(4 + j))
        rows = [(c3, s3) for s3 in s3set for c3 in range(4)]
        ones_idx = len(rows)
        for r, (c3, s3) in enumerate(rows):
            w3r[0:64, 32 * j + r] = eff3[:, c3 * S + s3]
            w3r[64, 32 * j + r] = b3[c3]
        w3r[64, 32 * j + ones_idx] = 1.0
        for g in range(2):
            s = 4 * g + j
            for r, (c3, s3) in enumerate(rows):
                k = s3 - s + 1
                if 0 <= k < 3:
                    w4g[g, 32 * j + r, :] = w4[:, c3, k]
            w4g[g, 32 * j + ones_idx, :] = b4

    return dict(
        w1a=w1e[:128].astype(np.float16),
        w1b=w1e[128:].astype(np.float16),
        w2e=w2e.astype(np.float16),
        w3r=w3r.astype(np.float16),
        w4g0=np.ascontiguousarray(w4g[0]).astype(np.float16),
        w4g1=np.ascontiguousarray(w4g[1]).astype(np.float16),
        wlt=np.ascontiguousarray(wl.T).astype(np.float16),
        b1v=np.repeat(b1, S).reshape(128, 1).astype(np.float32),
        b2v=b2v.reshape(65, 1).astype(np.float32),
    )


def make_in_maps(inputs):
    """Full-input dict -> per-core in_maps (host-side transpose + fp16)."""
    x = np.asarray(inputs["x"], np.float32).reshape(B, CS)
    xt = np.ascontiguousarray(x.astype(np.float16).T)       # [232, B]
    wmap = _host_weights(
        *[np.asarray(inputs[k], np.float32) for k in
          ("w1", "b1", "w2", "b2", "w3", "b3", "w4", "b4", "wl")])
    in_maps = []
    for i in range(NCORES):
        sl = slice(i * BPC, (i + 1) * BPC)
        m = {"xhi": np.ascontiguousarray(xt[:128, sl]),
             "xlo": np.ascontiguousarray(xt[128:, sl])}
        m.update(wmap)
        in_maps.append(m)
    return in_maps


_NC_CACHE = None


def kernel(x, w1, b1, w2, b2, w3, b3, w4, b4, wl, bl):
    global _NC_CACHE
    # bl is constant along the softmax axis -> cancels; intentionally unused.
    in_maps = make_in_maps(dict(x=x, w1=w1, b1=b1, w2=w2, b2=b2, w3=w3,
                                b3=b3, w4=w4, b4=b4, wl=wl))
    if _NC_CACHE is None:
        _NC_CACHE = _build_nc()
    nc = _NC_CACHE

    core_ids = list(range(NCORES))
    res = run_bass_kernel_spmd(nc, in_maps, core_ids)
    outs = []
    for i in range(NCORES):
        # [128, NCHUNK, 2, BC] level-2 partials -> sum pairs, divide, transpose
        u2 = res.results[i]["outu"].astype(np.float32).reshape(128, NCHUNK, 2, BC)
        d2 = res.results[i]["outd"].astype(np.float32).reshape(128, NCHUNK, 2, BC)
        uu = u2.sum(axis=2).reshape(128, BPC)
        dd = d2.sum(axis=2).reshape(128, BPC)
        outs.append((uu / dd).T)
    return np.ascontiguousarray(np.concatenate(outs, axis=0))
